# revision 11
# baseline (speedup 1.0000x reference)
"""Trainium2 Bass kernel for CSPFM-style pooled channel-attention broadcast.

Math (per batch b):
    d = max(x[b], spatial)                       # [C]
    e = mean(x[b], spatial)                      # [C]
    z = d outer d + e outer e                    # [C, C]
    y = softmax(z, axis=-1)
    f = alpha * (d @ y) + beta * (e @ y)         # [C]
      = ((alpha*d + beta*e) / rowsum(exp(z))) @ exp(z)
    out[b, c, :, :] = f[c]

(No max-subtraction in the softmax: z <= maxd^2 + maxe^2 < 30, so exp(z)
stays in f32/bf16 range trivially.)

Sharding: data-parallel over batch across 8 NeuronCores (4 batches/core).

The kernel is pure memory traffic at both ends (stream 32 MiB of x for the
pools, write 32 MiB of broadcast output) with a tiny C=512 attention in the
middle, so both ends run in fp16 (host converts x down, upcasts out) which
halves HBM traffic; stats stay in f32, attention weights in bf16 (PE's
native 16-bit), all ~1000x inside the 2e-2 gate.

The two pooling passes over x (max + sum, ~1 elem/cycle/partition each on
any engine) are the dominant compute; they are split DVE(max+2 sums) /
ACT(14 sums) to fit under the ~80us DMA floor.  ACT walks its sum chunks in
reverse order so the two engines never scan the same xt tile concurrently
(same-tile scans cost ~4.4us vs ~3us from SBUF contention).

Queues: sync = input DMAs only; gpsimd(SWDGE) = stat gathers + output DMA
triggers; ACT's scalar queue carries nothing, so neither scans nor input
issue ever sit behind an output DMA.  The stat row layout [2, C] needed for
the PE z-matmuls comes from one strided SBUF->SBUF gather DMA (de[128,8]
transposed), removing the PE-transpose/PSUM-copy chain from the critical
path.  Batches are software-pipelined (pool b+1 before attention of b).
"""

import os
import sys
from contextlib import ExitStack

import numpy as np

for _p in (
    "/opt/trn_rl_repo",
    "/root/.axon_site",
    "/root/.axon_site/_ro/trn_rl_repo",
    "/root/.axon_site/_ro/pypackages",
):
    if os.path.isdir(_p) and _p not in sys.path:
        sys.path.append(_p)

import concourse.bass as bass  # noqa: E402
import concourse.tile as tile  # noqa: E402
from concourse import bacc, masks, mybir  # noqa: E402
from concourse.bass_utils import run_bass_kernel_spmd  # noqa: E402

F32 = mybir.dt.float32
F16 = mybir.dt.float16
BF16 = mybir.dt.bfloat16
AX = mybir.AxisListType.X
AF = mybir.ActivationFunctionType

B, C, H, W = 32, 512, 64, 64
S = H * W                # 4096 spatial positions
NCORES = 8
BL = B // NCORES         # 4 batches per core
NCH = C // 128           # 4 channel chunks of 128
HALF = S // 2            # broadcast tile width


def _emit(tc, out, x, alpha, beta):
    nc = tc.nc
    with ExitStack() as ctx:
        const = ctx.enter_context(tc.tile_pool(name="const", bufs=1))
        xpool = ctx.enter_context(tc.tile_pool(name="xin", bufs=10))
        depool = ctx.enter_context(tc.tile_pool(name="de", bufs=4))
        vpool = ctx.enter_context(tc.tile_pool(name="vdve", bufs=2))
        epool = ctx.enter_context(tc.tile_pool(name="expt", bufs=8))
        bpool = ctx.enter_context(tc.tile_pool(name="bcast", bufs=6))
        small = ctx.enter_context(tc.tile_pool(name="small", bufs=4))
        spool = ctx.enter_context(tc.tile_pool(name="sb8", bufs=2))
        zpsum = ctx.enter_context(tc.tile_pool(name="zp", bufs=2, space="PSUM"))
        fpsum = ctx.enter_context(tc.tile_pool(name="fp", bufs=2, space="PSUM"))
        tpsum = ctx.enter_context(tc.tile_pool(name="tp", bufs=2, space="PSUM"))

        ident = const.tile([128, 128], F32)
        masks.make_identity(nc, ident[:])
        zeros16 = const.tile([128, HALF], F16)
        nc.vector.memset(zeros16[:], 0.0)
        # scratch sink for the ACT-engine pooling sums (never read)
        trash = const.tile([128, S], BF16)
        ab = const.tile([1, 2], F32)
        nc.sync.dma_start(ab[0:1, 0:1], alpha[:])
        nc.sync.dma_start(ab[0:1, 1:2], beta[:])
        ab_bc = const.tile([128, 2], F32)
        nc.gpsimd.partition_broadcast(ab_bc[:], ab[0:1, :])

        de_tiles = {}

        def pool(b):
            de = depool.tile([128, 2 * NCH], F32)
            de_tiles[b] = de
            xts = []
            for cc in range(NCH):
                xt = xpool.tile([128, S], F16)
                xts.append(xt)
                nc.sync.dma_start(xt[:], x[b, cc * 128:(cc + 1) * 128, :])
                nc.vector.reduce_max(de[:, cc:cc + 1], xt[:], axis=AX)
            # sums: 14 tiles on ACT (walked in reverse so ACT trails DVE on
            # different tiles), 2 on DVE - balances both engines at ~60us
            dve_sum = NCH - 1 if b % 2 == 0 else -1
            if dve_sum >= 0:
                nc.vector.reduce_sum(de[:, NCH + dve_sum:NCH + dve_sum + 1],
                                     xts[dve_sum][:], axis=AX)
            for cc in reversed(range(NCH)):
                if cc == dve_sum:
                    continue
                nc.scalar.activation(
                    trash[:], xts[cc][:], AF.Copy,
                    accum_out=de[:, NCH + cc:NCH + cc + 1],
                )
            # sum -> mean in place; the row gather below then carries means
            nc.vector.tensor_scalar_mul(de[:, NCH:2 * NCH],
                                        de[:, NCH:2 * NCH], 1.0 / S)

        def attn(b):
            de = de_tiles.pop(b)
            # ---- stats to row layout: vdve[0,:] = d row, vdve[1,:] = e row.
            # PE transpose [128,8]->[8,128], DVE copy to SBUF (DVE, so it
            # never queues behind ACT's sum scans), then a gather DMA on the
            # gpsimd queue lands d chunks on partition 0, e chunks on 1.
            tp = tpsum.tile([2 * NCH, 128], F32)
            nc.tensor.transpose(tp[:], de[:], ident[:])
            sb8 = spool.tile([2 * NCH, 128], F32)
            nc.vector.tensor_copy(sb8[:], tp[:])
            vdve = vpool.tile([2, C], F32)
            nc.gpsimd.dma_start(vdve[:], sb8[:])

            # g = alpha*d + beta*e  (combined matvec weight vector, per chunk)
            g = small.tile([128, NCH], F32)
            gt = small.tile([128, NCH], F32)
            nc.vector.tensor_scalar_mul(g[:], de[:, 0:NCH], ab_bc[:, 0:1])
            nc.vector.tensor_scalar_mul(gt[:], de[:, NCH:2 * NCH],
                                        ab_bc[:, 1:2])
            nc.vector.tensor_add(g[:], g[:], gt[:])

            # ---- z per row-chunk via one K=2 matmul; E = exp(z); h = g/rowsum
            h = small.tile([128, NCH], BF16)
            e_tiles = []
            for ic in range(NCH):
                zp = zpsum.tile([128, C], F32)
                nc.tensor.matmul(zp[:], vdve[:, ic * 128:(ic + 1) * 128],
                                 vdve[:, 0:C], start=True, stop=True)
                et = epool.tile([128, C], BF16)
                ssum = small.tile([128, 1], F32)
                nc.scalar.activation(et[:], zp[:], AF.Exp, bias=0.0,
                                     scale=1.0, accum_out=ssum[:])
                rs = small.tile([128, 1], F32)
                nc.vector.reciprocal(rs[:], ssum[:])
                nc.vector.tensor_mul(h[:, ic:ic + 1], g[:, ic:ic + 1], rs[:])
                e_tiles.append(et)

            # ---- f columns per j-chunk: f[j] = sum_i h[i] E[i, j] ----
            for jc in range(NCH):
                fp = fpsum.tile([128, 1], F32)
                for ic in range(NCH):
                    nc.tensor.matmul(
                        fp[:], e_tiles[ic][:, jc * 128:(jc + 1) * 128],
                        h[:, ic:ic + 1],
                        start=(ic == 0), stop=(ic == NCH - 1),
                    )
                fcol = small.tile([128, 1], F32)
                nc.vector.tensor_copy(fcol[:], fp[:])
                # broadcast f along the free axis on DVE (fp16 perf mode);
                # one DMA writes the half-S tile twice via a stride-0 AP,
                # triggered from the gpsimd (SWDGE) queue
                bc = bpool.tile([128, HALF], F16)
                nc.vector.tensor_scalar_add(bc[:], zeros16[:], fcol[:])
                nc.gpsimd.dma_start(
                    out[b, jc * 128:(jc + 1) * 128, :],
                    bc[:].unsqueeze(1).broadcast_to([128, 2, HALF]),
                )

        # software pipeline: keep the input stream ahead of attention work
        pool(0)
        pool(1)
        attn(0)
        pool(2)
        attn(1)
        pool(3)
        attn(2)
        attn(3)


_CACHE = {}
LAST_RESULTS = None


def _build():
    nc = bacc.Bacc("TRN2", target_bir_lowering=False, debug=False,
                   enable_asserts=False, num_devices=NCORES)
    x = nc.dram_tensor("x", [BL, C, S], F16, kind="ExternalInput").ap()
    alpha = nc.dram_tensor("alpha", [1], F32, kind="ExternalInput").ap()
    beta = nc.dram_tensor("beta", [1], F32, kind="ExternalInput").ap()
    out = nc.dram_tensor("out", [BL, C, S], F16, kind="ExternalOutput").ap()
    with tile.TileContext(nc) as tc:
        _emit(tc, out, x, alpha, beta)
    nc.compile()
    return nc


def kernel(x, alpha, beta, _trace=False):
    global LAST_RESULTS
    if "nc" not in _CACHE:
        _CACHE["nc"] = _build()
    nc = _CACHE["nc"]

    xs = np.ascontiguousarray(
        np.asarray(x, dtype=np.float32).reshape(B, C, S).astype(np.float16))
    a = np.ascontiguousarray(np.asarray(alpha, dtype=np.float32).reshape(1))
    bt = np.ascontiguousarray(np.asarray(beta, dtype=np.float32).reshape(1))
    in_maps = [
        {"x": xs[k * BL:(k + 1) * BL], "alpha": a, "beta": bt}
        for k in range(NCORES)
    ]
    res = run_bass_kernel_spmd(nc, in_maps, list(range(NCORES)), trace=_trace)
    LAST_RESULTS = res
    full = np.concatenate(
        [np.asarray(res.results[k]["out"]) for k in range(NCORES)], axis=0
    )
    return full.reshape(B, C, H, W).astype(np.float32)


# revision 14
# speedup vs baseline: 1.0316x; 1.0316x over previous
"""Trainium2 Bass kernel for CSPFM-style pooled channel-attention broadcast.

Math (per batch b):
    d = max(x[b], spatial)                       # [C]
    e = mean(x[b], spatial)                      # [C]
    z = d outer d + e outer e                    # [C, C]
    y = softmax(z, axis=-1)
    f = alpha * (d @ y) + beta * (e @ y)         # [C]
      = ((alpha*d + beta*e) / rowsum(exp(z))) @ exp(z)
    out[b, c, :, :] = f[c]

(No max-subtraction in the softmax: z <= maxd^2 + maxe^2 < 30, so exp(z)
stays in f32/bf16 range trivially.)

Sharding: data-parallel over batch across 8 NeuronCores (4 batches/core).

I/O runs in fp16 (host converts x down, upcasts out) halving HBM traffic;
stats stay f32, attention weights bf16 - all ~1000x inside the 2e-2 gate.

The two pooling passes over x (max + sum, ~1 elem/cycle/partition) dominate
compute: max on DVE, sum on ACT (Copy+accum with an f8 trash sink to halve
its SBUF writes - concurrent scans are SBUF-bandwidth sensitive).

The stat row layout [2, C] for the PE z-matmuls is built by two chained
DMAs (a 128x8 DMA-transpose on the scalar queue, then a partition gather on
the gpsimd queue) so no compute engine sits in that dependency chain.
Queues: sync = input only; scalar = stat transposes; gpsimd = gathers +
output triggers + init scalars.

Late batches' h/fcol/broadcast run on ACT instead of DVE: by then ACT has
finished its sum scans while DVE is still scanning, so the attention tail
does not queue behind DVE's remaining reduces.  Batches are
software-pipelined (pool b+1 emitted before attention of b).
"""

import os
import sys
from contextlib import ExitStack

import numpy as np

for _p in (
    "/opt/trn_rl_repo",
    "/root/.axon_site",
    "/root/.axon_site/_ro/trn_rl_repo",
    "/root/.axon_site/_ro/pypackages",
):
    if os.path.isdir(_p) and _p not in sys.path:
        sys.path.append(_p)

import concourse.bass as bass  # noqa: E402
import concourse.tile as tile  # noqa: E402
from concourse import bacc, masks, mybir  # noqa: E402
from concourse.bass_utils import run_bass_kernel_spmd  # noqa: E402

F32 = mybir.dt.float32
F16 = mybir.dt.float16
BF16 = mybir.dt.bfloat16
F8 = mybir.dt.float8e4
AX = mybir.AxisListType.X
AF = mybir.ActivationFunctionType

B, C, H, W = 32, 512, 64, 64
S = H * W                # 4096 spatial positions
NCORES = 8
BL = B // NCORES         # 4 batches per core
NCH = C // 128           # 4 channel chunks of 128
HALF = S // 2            # broadcast tile width


def _emit(tc, out, x, alpha, beta):
    nc = tc.nc
    with ExitStack() as ctx:
        const = ctx.enter_context(tc.tile_pool(name="const", bufs=1))
        xpool = ctx.enter_context(tc.tile_pool(name="xin", bufs=10))
        depool = ctx.enter_context(tc.tile_pool(name="de", bufs=4))
        spool = ctx.enter_context(tc.tile_pool(name="sb8", bufs=2))
        vpool = ctx.enter_context(tc.tile_pool(name="vdve", bufs=2))
        epool = ctx.enter_context(tc.tile_pool(name="expt", bufs=8))
        bpool = ctx.enter_context(tc.tile_pool(name="bcast", bufs=6))
        small = ctx.enter_context(tc.tile_pool(name="small", bufs=4))
        zpsum = ctx.enter_context(tc.tile_pool(name="zp", bufs=2, space="PSUM"))
        fpsum = ctx.enter_context(tc.tile_pool(name="fp", bufs=2, space="PSUM"))
        tpsum = ctx.enter_context(tc.tile_pool(name="tp", bufs=2, space="PSUM"))

        ident = const.tile([128, 128], F32)
        masks.make_identity(nc, ident[:])
        # scratch sink for the ACT-engine pooling sums (never read); f8 to
        # halve ACT's SBUF write bandwidth during the scan phase
        trash = const.tile([128, S], F8)
        zeros16 = const.tile([128, HALF], F16)
        ab = const.tile([1, 2], F32)
        ab_bc = const.tile([128, 2], F32)

        de_tiles = {}

        def pool(b):
            de = depool.tile([128, 2 * NCH], F32)
            de_tiles[b] = de
            xts = []
            for cc in range(NCH):
                xt = xpool.tile([128, S], F16)
                xts.append(xt)
                nc.sync.dma_start(xt[:], x[b, cc * 128:(cc + 1) * 128, :])
                nc.vector.reduce_max(de[:, cc:cc + 1], xt[:], axis=AX)
            # sums on ACT, walked in reverse so ACT trails DVE on different
            # tiles
            for cc in reversed(range(NCH)):
                nc.scalar.activation(
                    trash[:], xts[cc][:], AF.Copy,
                    accum_out=de[:, NCH + cc:NCH + cc + 1],
                )
            # sum -> mean in place; the row gather below then carries means
            nc.vector.tensor_scalar_mul(de[:, NCH:2 * NCH],
                                        de[:, NCH:2 * NCH], 1.0 / S)

        def attn(b, late):
            de = de_tiles.pop(b)
            # ---- stats to row layout: vdve[0,:] = d row, vdve[1,:] = e row.
            # PE transpose [128,8]->[8,128], PSUM->SBUF copy on whichever of
            # DVE/ACT is not scanning in this phase, then a partition gather
            # on the gpsimd queue.
            tp = tpsum.tile([2 * NCH, 128], F32)
            nc.tensor.transpose(tp[:], de[:], ident[:])
            sb8 = spool.tile([2 * NCH, 128], F32)
            if late:
                nc.scalar.copy(sb8[:], tp[:])
            else:
                nc.vector.tensor_copy(sb8[:], tp[:])
            vdve = vpool.tile([2, C], F32)
            nc.gpsimd.dma_start(vdve[:], sb8[:])

            # g = alpha*d + beta*e  (combined matvec weight vector, per chunk)
            g = small.tile([128, NCH], F32)
            gt = small.tile([128, NCH], F32)
            nc.vector.tensor_scalar_mul(g[:], de[:, 0:NCH], ab_bc[:, 0:1])
            nc.vector.tensor_scalar_mul(gt[:], de[:, NCH:2 * NCH],
                                        ab_bc[:, 1:2])
            nc.vector.tensor_add(g[:], g[:], gt[:])

            # ---- z per row-chunk via one K=2 matmul; E = exp(z); h = g/rowsum
            h = small.tile([128, NCH], BF16)
            e_tiles = []
            for ic in range(NCH):
                zp = zpsum.tile([128, C], F32)
                nc.tensor.matmul(zp[:], vdve[:, ic * 128:(ic + 1) * 128],
                                 vdve[:, 0:C], start=True, stop=True)
                et = epool.tile([128, C], BF16)
                ssum = small.tile([128, 1], F32)
                nc.scalar.activation(et[:], zp[:], AF.Exp, bias=0.0,
                                     scale=1.0, accum_out=ssum[:])
                rs = small.tile([128, 1], F32)
                nc.vector.reciprocal(rs[:], ssum[:])
                if late:
                    nc.scalar.mul(h[:, ic:ic + 1], g[:, ic:ic + 1], rs[:])
                else:
                    nc.vector.tensor_mul(h[:, ic:ic + 1], g[:, ic:ic + 1],
                                         rs[:])
                e_tiles.append(et)

            # ---- f columns per j-chunk: f[j] = sum_i h[i] E[i, j] ----
            for jc in range(NCH):
                fp = fpsum.tile([128, 1], F32)
                for ic in range(NCH):
                    nc.tensor.matmul(
                        fp[:], e_tiles[ic][:, jc * 128:(jc + 1) * 128],
                        h[:, ic:ic + 1],
                        start=(ic == 0), stop=(ic == NCH - 1),
                    )
                fcol = small.tile([128, 1], F32)
                bc = bpool.tile([128, HALF], F16)
                # early batches broadcast on DVE (ACT still scanning); late
                # batches on ACT (DVE still scanning, ACT already done)
                if late:
                    nc.scalar.copy(fcol[:], fp[:])
                    nc.scalar.activation(bc[:], zeros16[:], AF.Identity,
                                         bias=fcol[:], scale=1.0)
                else:
                    nc.vector.tensor_copy(fcol[:], fp[:])
                    nc.vector.tensor_scalar_add(bc[:], zeros16[:], fcol[:])
                # one DMA writes the half-S tile twice via a stride-0 AP,
                # triggered from the gpsimd (SWDGE) queue
                nc.gpsimd.dma_start(
                    out[b, jc * 128:(jc + 1) * 128, :],
                    bc[:].unsqueeze(1).broadcast_to([128, 2, HALF]),
                )

        # software pipeline: keep the input stream ahead of attention work
        pool(0)
        # init scalars off the sync queue head so input DMAs start at t=0
        nc.gpsimd.dma_start(ab[0:1, 0:1], alpha[:])
        nc.gpsimd.dma_start(ab[0:1, 1:2], beta[:])
        nc.gpsimd.partition_broadcast(ab_bc[:], ab[0:1, :])
        nc.vector.memset(zeros16[:], 0.0)
        pool(1)
        attn(0, late=False)
        pool(2)
        attn(1, late=False)
        pool(3)
        attn(2, late=True)
        attn(3, late=True)


_CACHE = {}
LAST_RESULTS = None


def _build():
    nc = bacc.Bacc("TRN2", target_bir_lowering=False, debug=False,
                   enable_asserts=False, num_devices=NCORES)
    x = nc.dram_tensor("x", [BL, C, S], F16, kind="ExternalInput").ap()
    alpha = nc.dram_tensor("alpha", [1], F32, kind="ExternalInput").ap()
    beta = nc.dram_tensor("beta", [1], F32, kind="ExternalInput").ap()
    out = nc.dram_tensor("out", [BL, C, S], F16, kind="ExternalOutput").ap()
    with tile.TileContext(nc) as tc:
        _emit(tc, out, x, alpha, beta)
    nc.compile()
    return nc


def kernel(x, alpha, beta, _trace=False):
    global LAST_RESULTS
    if "nc" not in _CACHE:
        _CACHE["nc"] = _build()
    nc = _CACHE["nc"]

    xs = np.ascontiguousarray(
        np.asarray(x, dtype=np.float32).reshape(B, C, S).astype(np.float16))
    a = np.ascontiguousarray(np.asarray(alpha, dtype=np.float32).reshape(1))
    bt = np.ascontiguousarray(np.asarray(beta, dtype=np.float32).reshape(1))
    in_maps = [
        {"x": xs[k * BL:(k + 1) * BL], "alpha": a, "beta": bt}
        for k in range(NCORES)
    ]
    res = run_bass_kernel_spmd(nc, in_maps, list(range(NCORES)), trace=_trace)
    LAST_RESULTS = res
    full = np.concatenate(
        [np.asarray(res.results[k]["out"]) for k in range(NCORES)], axis=0
    )
    return full.reshape(B, C, H, W).astype(np.float32)


# revision 18
# speedup vs baseline: 1.0896x; 1.0562x over previous
"""Trainium2 Bass kernel for CSPFM-style pooled channel-attention broadcast.

Math (per batch b):
    d = max(x[b], spatial)                       # [C]
    e = mean(x[b], spatial)                      # [C]
    z = d outer d + e outer e                    # [C, C]
    y = softmax(z, axis=-1)
    f = alpha * (d @ y) + beta * (e @ y)         # [C]
      = ((alpha*d + beta*e) / rowsum(exp(z))) @ exp(z)
    out[b, c, :, :] = f[c]

(No max-subtraction in the softmax: z <= maxd^2 + maxe^2 < 30, so exp(z)
stays in f32/bf16 range trivially.)

Sharding: data-parallel over batch across 8 NeuronCores (4 batches/core).

I/O runs in fp16 (host converts x down, upcasts out) halving HBM traffic;
stats stay f32, attention weights bf16 - all ~1000x inside the 2e-2 gate.

The two pooling passes over x (max + sum, ~1 elem/cycle/partition) dominate
compute: max on DVE, sum on ACT (Copy+accum with an f8 trash sink to halve
its SBUF writes - concurrent scans are SBUF-bandwidth sensitive).

The stat row layout [2, C] for the PE z-matmuls is built by two chained
DMAs (a 128x8 DMA-transpose on the scalar queue, then a partition gather on
the gpsimd queue) so no compute engine sits in that dependency chain.
Queues: sync = input only; scalar = stat transposes; gpsimd = gathers +
output triggers + init scalars.

Late batches' h/fcol/broadcast run on ACT instead of DVE: by then ACT has
finished its sum scans while DVE is still scanning, so the attention tail
does not queue behind DVE's remaining reduces.  Batches are
software-pipelined (pool b+1 emitted before attention of b).
"""

import os
import sys
from contextlib import ExitStack

import numpy as np

for _p in (
    "/opt/trn_rl_repo",
    "/root/.axon_site",
    "/root/.axon_site/_ro/trn_rl_repo",
    "/root/.axon_site/_ro/pypackages",
):
    if os.path.isdir(_p) and _p not in sys.path:
        sys.path.append(_p)

import concourse.bass as bass  # noqa: E402
import concourse.tile as tile  # noqa: E402
from concourse import bacc, masks, mybir  # noqa: E402
from concourse.bass_utils import run_bass_kernel_spmd  # noqa: E402

F32 = mybir.dt.float32
F16 = mybir.dt.float16
BF16 = mybir.dt.bfloat16
F8 = mybir.dt.float8e4
AX = mybir.AxisListType.X
AF = mybir.ActivationFunctionType

B, C, H, W = 32, 512, 64, 64
S = H * W                # 4096 spatial positions
NCORES = 8
BL = B // NCORES         # 4 batches per core
NCH = C // 128           # 4 channel chunks of 128
BCW = 512                # broadcast tile width (DMA replicates it 8x)
NREP = S // BCW


def _emit(tc, out, x, alpha, beta):
    nc = tc.nc
    with ExitStack() as ctx:
        const = ctx.enter_context(tc.tile_pool(name="const", bufs=1))
        xpool = ctx.enter_context(tc.tile_pool(name="xin", bufs=10))
        depool = ctx.enter_context(tc.tile_pool(name="de", bufs=4))
        spool = ctx.enter_context(tc.tile_pool(name="sb8", bufs=2))
        vpool = ctx.enter_context(tc.tile_pool(name="vdve", bufs=2))
        epool = ctx.enter_context(tc.tile_pool(name="expt", bufs=8))
        bpool = ctx.enter_context(tc.tile_pool(name="bcast", bufs=6))
        small = ctx.enter_context(tc.tile_pool(name="small", bufs=4))
        zpsum = ctx.enter_context(tc.tile_pool(name="zp", bufs=2, space="PSUM"))
        fpsum = ctx.enter_context(tc.tile_pool(name="fp", bufs=2, space="PSUM"))
        tpsum = ctx.enter_context(tc.tile_pool(name="tp", bufs=2, space="PSUM"))

        ident = const.tile([128, 128], F32)
        masks.make_identity(nc, ident[:])
        # scratch sink for the ACT-engine pooling sums (never read); f8 to
        # halve ACT's SBUF write bandwidth during the scan phase
        trash = const.tile([128, S], F8)
        zeros16 = const.tile([128, BCW], F16)
        ab = const.tile([1, 2], F32)
        ab_bc = const.tile([128, 2], F32)

        de_tiles = {}

        def pool(b):
            de = depool.tile([128, 2 * NCH], F32)
            de_tiles[b] = de
            xts = []
            for cc in range(NCH):
                xt = xpool.tile([128, S], F16)
                xts.append(xt)
                nc.sync.dma_start(xt[:], x[b, cc * 128:(cc + 1) * 128, :])
                nc.vector.reduce_max(de[:, cc:cc + 1], xt[:], axis=AX)
            # sums on ACT, walked in reverse so ACT trails DVE on different
            # tiles
            for cc in reversed(range(NCH)):
                nc.scalar.activation(
                    trash[:], xts[cc][:], AF.Copy,
                    accum_out=de[:, NCH + cc:NCH + cc + 1],
                )
            # sum -> mean in place; the row gather below then carries means
            nc.vector.tensor_scalar_mul(de[:, NCH:2 * NCH],
                                        de[:, NCH:2 * NCH], 1.0 / S)

        def attn(b, late):
            # the attention chain is latency work gating the output stream;
            # let it preempt later batches' scans on every engine as soon as
            # its deps are ready (scans are throughput work and backfill)
            with tc.high_priority():
                _attn(b, late)

        def _attn(b, late):
            de = de_tiles.pop(b)
            # ---- stats to row layout: vdve[0,:] = d row, vdve[1,:] = e row.
            # PE transpose [128,8]->[8,128], PSUM->SBUF copy on whichever of
            # DVE/ACT is not scanning in this phase, then a partition gather
            # on the gpsimd queue.
            tp = tpsum.tile([2 * NCH, 128], F32)
            nc.tensor.transpose(tp[:], de[:], ident[:])
            sb8 = spool.tile([2 * NCH, 128], F32)
            if late:
                nc.scalar.copy(sb8[:], tp[:])
            else:
                nc.vector.tensor_copy(sb8[:], tp[:])
            vdve = vpool.tile([2, C], F32)
            nc.gpsimd.dma_start(vdve[:], sb8[:])

            # g = alpha*d + beta*e  (combined matvec weight vector, per chunk)
            g = small.tile([128, NCH], F32)
            gt = small.tile([128, NCH], F32)
            nc.vector.tensor_scalar_mul(g[:], de[:, 0:NCH], ab_bc[:, 0:1])
            nc.vector.tensor_scalar_mul(gt[:], de[:, NCH:2 * NCH],
                                        ab_bc[:, 1:2])
            nc.vector.tensor_add(g[:], g[:], gt[:])

            # ---- z per row-chunk via one K=2 matmul; E = exp(z); h = g/rowsum
            h = small.tile([128, NCH], BF16)
            e_tiles = []
            for ic in range(NCH):
                zp = zpsum.tile([128, C], F32)
                nc.tensor.matmul(zp[:], vdve[:, ic * 128:(ic + 1) * 128],
                                 vdve[:, 0:C], start=True, stop=True)
                et = epool.tile([128, C], BF16)
                ssum = small.tile([128, 1], F32)
                nc.scalar.activation(et[:], zp[:], AF.Exp, bias=0.0,
                                     scale=1.0, accum_out=ssum[:])
                rs = small.tile([128, 1], F32)
                nc.vector.reciprocal(rs[:], ssum[:])
                if late:
                    nc.scalar.mul(h[:, ic:ic + 1], g[:, ic:ic + 1], rs[:])
                else:
                    nc.vector.tensor_mul(h[:, ic:ic + 1], g[:, ic:ic + 1],
                                         rs[:])
                e_tiles.append(et)

            # ---- f columns per j-chunk: f[j] = sum_i h[i] E[i, j] ----
            for jc in range(NCH):
                fp = fpsum.tile([128, 1], F32)
                for ic in range(NCH):
                    nc.tensor.matmul(
                        fp[:], e_tiles[ic][:, jc * 128:(jc + 1) * 128],
                        h[:, ic:ic + 1],
                        start=(ic == 0), stop=(ic == NCH - 1),
                    )
                fcol = small.tile([128, 1], F32)
                bc = bpool.tile([128, BCW], F16)
                # early batches broadcast on DVE (ACT still scanning); late
                # batches on ACT (DVE still scanning, ACT already done)
                if late:
                    nc.scalar.copy(fcol[:], fp[:])
                    nc.scalar.activation(bc[:], zeros16[:], AF.Identity,
                                         bias=fcol[:], scale=1.0)
                else:
                    nc.vector.tensor_copy(fcol[:], fp[:])
                    nc.vector.tensor_scalar_add(bc[:], zeros16[:], fcol[:])
                # one DMA writes the small tile NREP times via a stride-0 AP,
                # triggered from the gpsimd (SWDGE) queue
                nc.gpsimd.dma_start(
                    out[b, jc * 128:(jc + 1) * 128, :],
                    bc[:].unsqueeze(1).broadcast_to([128, NREP, BCW]),
                )

        # software pipeline: keep the input stream ahead of attention work
        pool(0)
        # init scalars off the sync queue head so input DMAs start at t=0
        nc.gpsimd.dma_start(ab[0:1, 0:1], alpha[:])
        nc.gpsimd.dma_start(ab[0:1, 1:2], beta[:])
        nc.gpsimd.partition_broadcast(ab_bc[:], ab[0:1, :])
        nc.vector.memset(zeros16[:], 0.0)
        pool(1)
        attn(0, late=False)
        pool(2)
        attn(1, late=False)
        pool(3)
        attn(2, late=True)
        attn(3, late=True)


_CACHE = {}
LAST_RESULTS = None


def _build():
    nc = bacc.Bacc("TRN2", target_bir_lowering=False, debug=False,
                   enable_asserts=False, num_devices=NCORES)
    x = nc.dram_tensor("x", [BL, C, S], F16, kind="ExternalInput").ap()
    alpha = nc.dram_tensor("alpha", [1], F32, kind="ExternalInput").ap()
    beta = nc.dram_tensor("beta", [1], F32, kind="ExternalInput").ap()
    out = nc.dram_tensor("out", [BL, C, S], F16, kind="ExternalOutput").ap()
    with tile.TileContext(nc) as tc:
        _emit(tc, out, x, alpha, beta)
    nc.compile()
    return nc


def kernel(x, alpha, beta, _trace=False):
    global LAST_RESULTS
    if "nc" not in _CACHE:
        _CACHE["nc"] = _build()
    nc = _CACHE["nc"]

    xs = np.ascontiguousarray(
        np.asarray(x, dtype=np.float32).reshape(B, C, S).astype(np.float16))
    a = np.ascontiguousarray(np.asarray(alpha, dtype=np.float32).reshape(1))
    bt = np.ascontiguousarray(np.asarray(beta, dtype=np.float32).reshape(1))
    in_maps = [
        {"x": xs[k * BL:(k + 1) * BL], "alpha": a, "beta": bt}
        for k in range(NCORES)
    ]
    res = run_bass_kernel_spmd(nc, in_maps, list(range(NCORES)), trace=_trace)
    LAST_RESULTS = res
    full = np.concatenate(
        [np.asarray(res.results[k]["out"]) for k in range(NCORES)], axis=0
    )
    return full.reshape(B, C, H, W).astype(np.float32)


# revision 23
# speedup vs baseline: 1.1200x; 1.0279x over previous
"""Trainium2 Bass kernel for CSPFM-style pooled channel-attention broadcast.

Math (per batch b):
    d = max(x[b], spatial)                       # [C]
    e = mean(x[b], spatial)                      # [C]
    z = d outer d + e outer e                    # [C, C]
    y = softmax(z, axis=-1)
    f = alpha * (d @ y) + beta * (e @ y)         # [C]
      = ((alpha*d + beta*e) / rowsum(exp(z))) @ exp(z)
    out[b, c, :, :] = f[c]

(No max-subtraction in the softmax: z <= maxd^2 + maxe^2 < 30, so exp(z)
stays in f32/bf16 range trivially.)

Sharding: data-parallel over batch across 8 NeuronCores (4 batches/core).

I/O runs in fp16 (host converts x down, upcasts out) halving HBM traffic;
stats stay f32, attention weights bf16 - all ~1000x inside the 2e-2 gate.

The two pooling passes over x dominate compute: a [128,4096] fp16 scan
costs ~4.4us on DVE / ~3.7us on ACT (1 elem/cycle/partition, no 16-bit
speedup for reductions), so max rides DVE (the only reducer with a max op)
and the sums ride ACT (Copy+accum with an f8 trash sink).

The per-batch attention chain is latency work gating the output stream; it
runs under tc.high_priority() so each hop preempts the scan backlog, and
the per-chunk reciprocal/h/fcol ops are batched into one op per batch to
minimize cross-engine preemption waits.  Queues: sync = inputs + half the
output triggers, scalar = other output triggers + nothing else, gpsimd =
stat gathers.  Output DMAs replicate a [128, 2048] fp16 tile twice via a
stride-0 AP.  Batches are software-pipelined (pool b+1 before attn b).
"""

import os
import sys
from contextlib import ExitStack

import numpy as np

for _p in (
    "/opt/trn_rl_repo",
    "/root/.axon_site",
    "/root/.axon_site/_ro/trn_rl_repo",
    "/root/.axon_site/_ro/pypackages",
):
    if os.path.isdir(_p) and _p not in sys.path:
        sys.path.append(_p)

import concourse.bass as bass  # noqa: E402
import concourse.tile as tile  # noqa: E402
from concourse import bacc, masks, mybir  # noqa: E402
from concourse.bass_utils import run_bass_kernel_spmd  # noqa: E402

F32 = mybir.dt.float32
F16 = mybir.dt.float16
BF16 = mybir.dt.bfloat16
F8 = mybir.dt.float8e4
AX = mybir.AxisListType.X
AF = mybir.ActivationFunctionType

B, C, H, W = 32, 512, 64, 64
S = H * W                # 4096 spatial positions
NCORES = 8
BL = B // NCORES         # 4 batches per core
NCH = C // 128           # 4 channel chunks of 128
BCW = 2048               # broadcast tile width (DMA replicates it 2x)
NREP = S // BCW


def _emit(tc, out, x, alpha, beta):
    nc = tc.nc
    with ExitStack() as ctx:
        const = ctx.enter_context(tc.tile_pool(name="const", bufs=1))
        xpool = ctx.enter_context(tc.tile_pool(name="xin", bufs=10))
        depool = ctx.enter_context(tc.tile_pool(name="de", bufs=4))
        spool = ctx.enter_context(tc.tile_pool(name="sb8", bufs=2))
        vpool = ctx.enter_context(tc.tile_pool(name="vdve", bufs=2))
        epool = ctx.enter_context(tc.tile_pool(name="expt", bufs=8))
        bpool = ctx.enter_context(tc.tile_pool(name="bcast", bufs=6))
        small = ctx.enter_context(tc.tile_pool(name="small", bufs=4))
        zpsum = ctx.enter_context(tc.tile_pool(name="zp", bufs=2, space="PSUM"))
        fpsum = ctx.enter_context(tc.tile_pool(name="fp", bufs=2, space="PSUM"))
        tpsum = ctx.enter_context(tc.tile_pool(name="tp", bufs=2, space="PSUM"))

        ident = const.tile([128, 128], F32)
        masks.make_identity(nc, ident[:])
        # scratch sink for the ACT-engine pooling sums (never read); f8 to
        # halve ACT's SBUF write bandwidth during the scan phase
        trash = const.tile([128, S], F8)
        zeros16 = const.tile([128, BCW], F16)
        ab = const.tile([1, 2], F32)
        ab_bc = const.tile([128, 2], F32)

        de_tiles = {}

        def pool(b):
            de = depool.tile([128, 2 * NCH], F32)
            de_tiles[b] = de
            for cc in range(NCH):
                xt = xpool.tile([128, S], F16)
                nc.sync.dma_start(xt[:], x[b, cc * 128:(cc + 1) * 128, :])
                nc.vector.reduce_max(de[:, cc:cc + 1], xt[:], axis=AX)
                nc.scalar.activation(
                    trash[:], xt[:], AF.Copy,
                    accum_out=de[:, NCH + cc:NCH + cc + 1],
                )
            # sum -> mean in place; the row gather below then carries means
            nc.vector.tensor_scalar_mul(de[:, NCH:2 * NCH],
                                        de[:, NCH:2 * NCH], 1.0 / S)

        def attn(b, late):
            # latency chain gating the output stream: preempt the scan
            # backlog on every engine as soon as deps are ready
            with tc.high_priority():
                bcs = _attn(b, late)
            # output triggers at normal priority, alternating HWDGE queues
            for jc, bc in enumerate(bcs):
                eng = nc.sync if jc % 2 == 0 else nc.scalar
                eng.dma_start(
                    out[b, jc * 128:(jc + 1) * 128, :],
                    bc[:].unsqueeze(1).broadcast_to([128, NREP, BCW]),
                )

        def _attn(b, late):
            de = de_tiles.pop(b)
            # ---- stats to row layout: vdve[0,:] = d row, vdve[1,:] = e row.
            # PE transpose [128,8]->[8,128], PSUM->SBUF copy, then a
            # partition gather on the gpsimd queue.
            tp = tpsum.tile([2 * NCH, 128], F32)
            nc.tensor.transpose(tp[:], de[:], ident[:])
            sb8 = spool.tile([2 * NCH, 128], F32)
            if late:
                nc.scalar.copy(sb8[:], tp[:])
            else:
                nc.vector.tensor_copy(sb8[:], tp[:])
            vdve = vpool.tile([2, C], F32)
            nc.gpsimd.dma_start(vdve[:], sb8[:])

            # g = alpha*d + beta*e  (combined matvec weight vector, per chunk)
            g = small.tile([128, NCH], F32)
            gt = small.tile([128, NCH], F32)
            nc.vector.tensor_scalar_mul(g[:], de[:, 0:NCH], ab_bc[:, 0:1])
            nc.vector.tensor_scalar_mul(gt[:], de[:, NCH:2 * NCH],
                                        ab_bc[:, 1:2])
            nc.vector.tensor_add(g[:], g[:], gt[:])

            # ---- z per row-chunk via one K=2 matmul; E = exp(z) ----
            ssums = small.tile([128, NCH], F32)
            e_tiles = []
            for ic in range(NCH):
                zp = zpsum.tile([128, C], F32)
                nc.tensor.matmul(zp[:], vdve[:, ic * 128:(ic + 1) * 128],
                                 vdve[:, 0:C], start=True, stop=True)
                et = epool.tile([128, C], BF16)
                nc.scalar.activation(et[:], zp[:], AF.Exp, bias=0.0,
                                     scale=1.0,
                                     accum_out=ssums[:, ic:ic + 1])
                e_tiles.append(et)
            # h = g / rowsum, one op per batch (minimises preemption waits)
            rs = small.tile([128, NCH], F32)
            nc.vector.reciprocal(rs[:], ssums[:])
            h = small.tile([128, NCH], BF16)
            nc.vector.tensor_mul(h[:], g[:], rs[:])

            # ---- f columns per j-chunk into one PSUM tile ----
            fp = fpsum.tile([128, NCH], F32)
            for jc in range(NCH):
                for ic in range(NCH):
                    nc.tensor.matmul(
                        fp[:, jc:jc + 1],
                        e_tiles[ic][:, jc * 128:(jc + 1) * 128],
                        h[:, ic:ic + 1],
                        start=(ic == 0), stop=(ic == NCH - 1),
                    )
            fcol = small.tile([128, NCH], F32)
            if late:
                nc.scalar.copy(fcol[:], fp[:])
            else:
                nc.vector.tensor_copy(fcol[:], fp[:])
            bcs = []
            for jc in range(NCH):
                bc = bpool.tile([128, BCW], F16)
                if late:
                    nc.scalar.activation(bc[:], zeros16[:], AF.Identity,
                                         bias=fcol[:, jc:jc + 1], scale=1.0)
                else:
                    nc.vector.tensor_scalar_add(bc[:], zeros16[:],
                                                fcol[:, jc:jc + 1])
                bcs.append(bc)
            return bcs

        # software pipeline: keep the input stream ahead of attention work
        pool(0)
        # init scalars off the sync queue head so input DMAs start at t=0
        nc.gpsimd.dma_start(ab[0:1, 0:1], alpha[:])
        nc.gpsimd.dma_start(ab[0:1, 1:2], beta[:])
        nc.gpsimd.partition_broadcast(ab_bc[:], ab[0:1, :])
        nc.vector.memset(zeros16[:], 0.0)
        pool(1)
        attn(0, late=False)
        pool(2)
        attn(1, late=False)
        pool(3)
        attn(2, late=True)
        attn(3, late=True)


_CACHE = {}
LAST_RESULTS = None


def _build():
    nc = bacc.Bacc("TRN2", target_bir_lowering=False, debug=False,
                   enable_asserts=False, num_devices=NCORES)
    x = nc.dram_tensor("x", [BL, C, S], F16, kind="ExternalInput").ap()
    alpha = nc.dram_tensor("alpha", [1], F32, kind="ExternalInput").ap()
    beta = nc.dram_tensor("beta", [1], F32, kind="ExternalInput").ap()
    out = nc.dram_tensor("out", [BL, C, S], F16, kind="ExternalOutput").ap()
    with tile.TileContext(nc) as tc:
        _emit(tc, out, x, alpha, beta)
    nc.compile()
    return nc


def kernel(x, alpha, beta, _trace=False):
    global LAST_RESULTS
    if "nc" not in _CACHE:
        _CACHE["nc"] = _build()
    nc = _CACHE["nc"]

    xs = np.ascontiguousarray(
        np.asarray(x, dtype=np.float32).reshape(B, C, S).astype(np.float16))
    a = np.ascontiguousarray(np.asarray(alpha, dtype=np.float32).reshape(1))
    bt = np.ascontiguousarray(np.asarray(beta, dtype=np.float32).reshape(1))
    in_maps = [
        {"x": xs[k * BL:(k + 1) * BL], "alpha": a, "beta": bt}
        for k in range(NCORES)
    ]
    res = run_bass_kernel_spmd(nc, in_maps, list(range(NCORES)), trace=_trace)
    LAST_RESULTS = res
    full = np.concatenate(
        [np.asarray(res.results[k]["out"]) for k in range(NCORES)], axis=0
    )
    return full.reshape(B, C, H, W).astype(np.float32)
